# revision 8
# baseline (speedup 1.0000x reference)
"""Causal self-attention (GQA, RoPE, QK-RMSNorm) Trainium2 Bass kernel.

Sharding: 8 cores = 2 batches x 4 KV groups. Core i handles batch i//4 and
KV group i%4 (4 query heads + 1 KV head). c_q/c_k/c_v column-sharded,
c_proj row-sharded; the partial output sums are reduced on the host.

Device-side layout trick: the host ships x^T (plus stacked cos/sin tables),
so every matmul contraction dim lands on SBUF partitions with zero on-device
transposes of x. Attention uses the "scores-transposed" formulation:
  s^T[tk, tq] = k^T.T @ q^T  (k block stationary)
  p^T = exp(s^T * scale)     (no max subtraction: QK-RMSNorm bounds |s*scale| <= sqrt(128))
  y[tq, d], r[tq] = p^T.T @ [v | 1]  (ones column gives the softmax denominator)
so softmax needs no partition-dim reductions and no probability transposes.
"""

import sys

sys.path.insert(0, "/opt/trn_rl_repo")

import numpy as np

import concourse.bacc as bacc
import concourse.tile as tile
import concourse.mybir as mybir
from concourse.bass_utils import run_bass_kernel_spmd

# Problem constants (hardcoded per contract)
B = 2
T = 2048
D = 2048
N_HEAD = 16
N_KV = 4
DH = 128
REP = N_HEAD // N_KV  # 4 query heads per KV head
HG = REP * DH  # 512 query columns per core
EPS = 1.1920928955078125e-07
SCALE = 1.0 / float(np.sqrt(DH))
MASK_VAL = -1e9

P = 128
TCH = 512  # T chunk (psum free dim)
NTCH = T // TCH  # 4
NDCH = D // P  # 16
NTB = T // P  # 16 Tk blocks

F32 = mybir.dt.float32
F32R = mybir.dt.float32r
BF16 = mybir.dt.bfloat16

N_CORES = 8

_CACHE = {}


def _build():
    nc = bacc.Bacc("TRN2", num_devices=N_CORES)
    xT = nc.dram_tensor("xT", [D, T], F32, kind="ExternalInput").ap()
    cos2 = nc.dram_tensor("cos2", [P, T], F32, kind="ExternalInput").ap()
    sin2n = nc.dram_tensor("sin2n", [P, T], F32, kind="ExternalInput").ap()
    wq = nc.dram_tensor("wq", [D, HG], F32, kind="ExternalInput").ap()
    wk = nc.dram_tensor("wk", [D, DH], F32, kind="ExternalInput").ap()
    wv = nc.dram_tensor("wv", [D, DH], F32, kind="ExternalInput").ap()
    wo = nc.dram_tensor("wo", [HG, D], F32, kind="ExternalInput").ap()
    out = nc.dram_tensor("out", [T, D], F32, kind="ExternalOutput").ap()

    with tile.TileContext(nc) as tc:
        _trace(tc, xT, cos2, sin2n, wq, wk, wv, wo, out)
    nc.compile()
    return nc


def _trace(tc, xT, cos2, sin2n, wq, wk, wv, wo, out):
    nc = tc.nc
    from contextlib import ExitStack

    ctx = ExitStack()
    with ctx:
        # ---------------- persistent pools ----------------
        const_pool = ctx.enter_context(tc.tile_pool(name="consts", bufs=1))
        attn_pool = ctx.enter_context(tc.tile_pool(name="attn", bufs=1))

        # constants
        ident_f = const_pool.tile([P, P], F32)
        nc.gpsimd.memset(ident_f, 0.0)
        nc.gpsimd.affine_select(
            out=ident_f, in_=ident_f, compare_op=mybir.AluOpType.not_equal,
            fill=1.0, base=0, pattern=[[-1, P]], channel_multiplier=1,
        )
        ident = const_pool.tile([P, P], F32R)
        nc.vector.tensor_copy(out=ident, in_=ident_f)

        onesm_f = const_pool.tile([P, P], F32)
        nc.vector.memset(onesm_f, 1.0)
        onesm = const_pool.tile([P, P], F32R)
        nc.vector.tensor_copy(out=onesm, in_=onesm_f)

        # additive causal mask for a diagonal 128x128 block: keep iff col >= row
        mask_sb = const_pool.tile([P, P], F32)
        nc.gpsimd.memset(mask_sb, 0.0)
        nc.gpsimd.affine_select(
            out=mask_sb, in_=mask_sb, compare_op=mybir.AluOpType.is_ge,
            fill=MASK_VAL, base=0, pattern=[[1, P]], channel_multiplier=-1,
        )

        eps_sb = const_pool.tile([P, 1], F32)
        nc.vector.memset(eps_sb, EPS)

        # final attention operands (fp32r / bf16)
        qrot = [attn_pool.tile([P, T], F32R, tag=f"qrot{h}", name=f"qrot{h}")
                for h in range(REP)]
        krot = attn_pool.tile([P, T], F32R, tag="krot", name="krot")
        vaug = [attn_pool.tile([P, DH + 1], BF16, tag=f"vaug{m}", name=f"vaug{m}")
                for m in range(NTB)]
        # ---------------- phases A-C: projections + RoPE/RMS + v prep ----------------
        # fused per T-chunk: evacuate proj psums into chunk tiles, do RMS stats
        # (all-ones matmul -> replicated column sums), RoPE via half-swap DMA,
        # and v transposes, all on [128, 512] slices.
        with tc.tile_pool(name="projw", bufs=1) as wpool, \
             tc.tile_pool(name="xts", bufs=2) as xpool, \
             tc.tile_pool(name="pre", bufs=2) as prepool, \
             tc.tile_pool(name="ps_proj", bufs=1, space="PSUM") as ps_proj, \
             tc.tile_pool(name="ps_rstd", bufs=1, space="PSUM") as ps_rstd:

            wq_sb = wpool.tile([P, NDCH, HG], F32R, name="wq_sb")
            wk_sb = wpool.tile([P, NDCH, DH], F32R, name="wk_sb")
            wv_sb = wpool.tile([P, NDCH, DH], F32R, name="wv_sb")
            nc.sync.dma_start(
                out=wq_sb, in_=wq.rearrange("(n p) h -> p n h", p=P).bitcast(F32R))
            nc.sync.dma_start(
                out=wk_sb, in_=wk.rearrange("(n p) h -> p n h", p=P).bitcast(F32R))
            nc.sync.dma_start(
                out=wv_sb, in_=wv.rearrange("(n p) h -> p n h", p=P).bitcast(F32R))

            cos2_sb = wpool.tile([P, T], F32, name="cos2_sb")
            sin2n_sb = wpool.tile([P, T], F32, name="sin2n_sb")
            nc.sync.dma_start(out=cos2_sb, in_=cos2)
            nc.sync.dma_start(out=sin2n_sb, in_=sin2n)

            for j in range(NTCH):
                cs = slice(j * TCH, (j + 1) * TCH)
                psums = [ps_proj.tile([P, TCH], F32, tag=f"proj{i}", name=f"proj{i}_{j}")
                         for i in range(6)]
                for i in range(NDCH):
                    xt = xpool.tile([P, TCH], F32R, tag="xt", name=f"xt_{j}_{i}")
                    nc.sync.dma_start(
                        out=xt,
                        in_=xT[i * P:(i + 1) * P, j * TCH:(j + 1) * TCH].bitcast(F32R),
                    )
                    for h in range(REP):
                        nc.tensor.matmul(
                            psums[h], wq_sb[:, i, h * DH:(h + 1) * DH], xt,
                            start=(i == 0), stop=(i == NDCH - 1))
                    nc.tensor.matmul(psums[4], wk_sb[:, i, :], xt,
                                     start=(i == 0), stop=(i == NDCH - 1))
                    nc.tensor.matmul(psums[5], wv_sb[:, i, :], xt,
                                     start=(i == 0), stop=(i == NDCH - 1))

                # q/k heads: copy out, RMS stats, RoPE, write fp32r chunk
                for i in range(5):
                    dst = qrot[i] if i < REP else krot
                    pre = prepool.tile([P, TCH], F32, tag="pre", name=f"pre{i}_{j}")
                    nc.scalar.copy(out=pre, in_=psums[i])
                    sq = prepool.tile([P, TCH], F32R, tag="sq", name=f"sq{i}_{j}")
                    nc.vector.tensor_mul(sq, pre, pre)
                    rps = ps_rstd.tile([P, TCH], F32, tag="rstd", name=f"rstd{i}_{j}")
                    nc.tensor.matmul(rps, onesm, sq, start=True, stop=True)
                    srt = prepool.tile([P, TCH], F32, tag="srt", name=f"srt{i}_{j}")
                    nc.scalar.activation(
                        out=srt, in_=rps,
                        func=mybir.ActivationFunctionType.Sqrt,
                        scale=1.0 / DH, bias=eps_sb)
                    nc.vector.reciprocal(out=srt, in_=srt)
                    swp = prepool.tile([P, TCH], F32, tag="swp", name=f"swp{i}_{j}")
                    nc.sync.dma_start(out=swp[0:64, :], in_=pre[64:128, :])
                    nc.sync.dma_start(out=swp[64:128, :], in_=pre[0:64, :])
                    m1 = prepool.tile([P, TCH], F32, tag="m1", name=f"m1_{i}_{j}")
                    nc.vector.tensor_mul(m1, pre, cos2_sb[:, cs])
                    m2 = prepool.tile([P, TCH], F32, tag="m2", name=f"m2_{i}_{j}")
                    nc.vector.tensor_mul(m2, swp, sin2n_sb[:, cs])
                    nc.vector.tensor_add(m1, m1, m2)
                    nc.vector.tensor_mul(dst[:, cs], m1, srt)

                # v: copy chunk then transpose its 4 blocks into vaug tiles
                vsb = prepool.tile([P, TCH], F32R, tag="vsb", name=f"vsb{j}")
                nc.scalar.copy(out=vsb, in_=psums[5])
                for mm in range(4):
                    m = 4 * j + mm
                    tr = ps_rstd.tile([P, P], F32R, tag="vtr", name=f"vtr{m}")
                    nc.tensor.transpose(tr, vsb[:, mm * P:(mm + 1) * P], ident)
                    nc.scalar.copy(out=vaug[m][:, 0:DH], in_=tr)
                    nc.vector.memset(vaug[m][:, DH:DH + 1], 1.0)

        pt_pool = ctx.enter_context(tc.tile_pool(name="pt", bufs=1))
        yt_pool = ctx.enter_context(tc.tile_pool(name="yt", bufs=2))
        osb_pool = ctx.enter_context(tc.tile_pool(name="osb", bufs=3))
        ps_attn = ctx.enter_context(tc.tile_pool(name="ps_attn", bufs=2, space="PSUM"))
        ps_y = ctx.enter_context(tc.tile_pool(name="ps_y", bufs=2, space="PSUM"))

        # wo loads into space freed by the projection pools
        wo_pool = ctx.enter_context(tc.tile_pool(name="wop", bufs=1))
        wo_sb = wo_pool.tile([P, REP, D], F32R, name="wo_sb")
        nc.sync.dma_start(
            out=wo_sb,
            in_=wo.rearrange("(n p) d -> p n d", p=P).bitcast(F32R),
        )

        # ---------------- phase D: attention + output projection ----------------
        for j in range(NTCH):
            qs = slice(j * TCH, (j + 1) * TCH)
            yt = [yt_pool.tile([P, TCH], F32R, tag=f"yt{h}", name=f"yt{h}_{j}")
                  for h in range(REP)]
            for h in range(REP):
                pts = [pt_pool.tile([P, TCH], BF16, tag=f"pt{m}", name=f"pt{m}_{j}_{h}")
                       for m in range(4 * j + 4)]
                for m in range(4 * j + 4):
                    sps = ps_attn.tile([P, TCH], F32, tag="s", name=f"s{j}_{h}_{m}")
                    nc.tensor.matmul(sps, krot[:, m * P:(m + 1) * P], qrot[h][:, qs],
                                     start=True, stop=True)
                    if m >= 4 * j:
                        dcol = P * (m - 4 * j)
                        ds_ = slice(dcol, dcol + P)
                        nc.vector.tensor_add(sps[:, ds_], sps[:, ds_], mask_sb)
                        if dcol > 0:
                            nc.vector.memset(pts[m][:, 0:dcol], 0.0)
                        nc.scalar.activation(
                            out=pts[m][:, dcol:TCH], in_=sps[:, dcol:TCH],
                            func=mybir.ActivationFunctionType.Exp, scale=SCALE)
                    else:
                        nc.scalar.activation(
                            out=pts[m], in_=sps,
                            func=mybir.ActivationFunctionType.Exp, scale=SCALE)
                for n in range(4):
                    last = 4 * j + n
                    yps = ps_y.tile([P, DH + 1], F32, tag="y", name=f"y{j}_{h}_{n}")
                    for m in range(last + 1):
                        nc.tensor.matmul(yps, pts[m][:, n * P:(n + 1) * P], vaug[m],
                                         start=(m == 0), stop=(m == last))
                    rinv = osb_pool.tile([P, 1], F32, tag="rinv", name=f"rinv{j}{h}{n}")
                    nc.vector.reciprocal(out=rinv, in_=yps[:, DH:DH + 1])
                    ynorm = osb_pool.tile([P, P], F32R, tag="ynorm",
                                          name=f"ynorm{j}{h}{n}")
                    nc.vector.tensor_scalar_mul(ynorm, yps[:, 0:DH], rinv)
                    ytr = ps_y.tile([P, P], F32R, tag="ytr", name=f"ytr{j}{h}{n}")
                    nc.tensor.transpose(ytr, ynorm, ident)
                    nc.scalar.copy(out=yt[h][:, n * P:(n + 1) * P], in_=ytr)
            for n in range(4):
                for dc in range(NTCH):
                    ops = ps_attn.tile([P, TCH], F32, tag="o", name=f"o{j}_{n}_{dc}")
                    for h in range(REP):
                        nc.tensor.matmul(
                            ops, yt[h][:, n * P:(n + 1) * P],
                            wo_sb[:, h, dc * TCH:(dc + 1) * TCH],
                            start=(h == 0), stop=(h == REP - 1))
                    osb = osb_pool.tile([P, TCH], F32, tag="osb", name=f"osb{j}{n}{dc}")
                    nc.any.tensor_copy(out=osb, in_=ops)
                    nc.sync.dma_start(
                        out=out[j * TCH + n * P: j * TCH + (n + 1) * P,
                                dc * TCH:(dc + 1) * TCH],
                        in_=osb)


def _prep_inputs(x, cos, sin, Wq, Wk, Wv, Wo):
    cosT = np.ascontiguousarray(cos[0, :, 0, :].T.astype(np.float32))  # [64, T]
    sinT = np.ascontiguousarray(sin[0, :, 0, :].T.astype(np.float32))
    cos2 = np.concatenate([cosT, cosT], axis=0)
    sin2n = np.concatenate([sinT, -sinT], axis=0)
    in_maps = []
    for i in range(N_CORES):
        b, g = i // 4, i % 4
        in_maps.append({
            "xT": np.ascontiguousarray(x[b].T.astype(np.float32)),
            "cos2": cos2,
            "sin2n": sin2n,
            "wq": np.ascontiguousarray(Wq[:, g * HG:(g + 1) * HG].astype(np.float32)),
            "wk": np.ascontiguousarray(Wk[:, g * DH:(g + 1) * DH].astype(np.float32)),
            "wv": np.ascontiguousarray(Wv[:, g * DH:(g + 1) * DH].astype(np.float32)),
            "wo": np.ascontiguousarray(Wo[g * HG:(g + 1) * HG, :].astype(np.float32)),
        })
    return in_maps


def bench(x, cos, sin, Wq, Wk, Wv, Wo, iters=20):
    """Device-resident timing of the compiled NEFF via the PJRT path.

    Stages all inputs (and fresh donated output buffers) on the devices
    before each timed call, so the measured wall time is dispatch + execute
    + sync only.
    """
    import time

    import jax
    from jax.sharding import Mesh, PartitionSpec
    from jax.experimental.shard_map import shard_map
    import concourse.bass2jax as bass2jax
    import concourse.mybir as mybir_

    if "nc" not in _CACHE:
        _CACHE["nc"] = _build()
    nc = _CACHE["nc"]
    in_maps = _prep_inputs(
        np.asarray(x), np.asarray(cos), np.asarray(sin),
        np.asarray(Wq), np.asarray(Wk), np.asarray(Wv), np.asarray(Wo))

    bass2jax.install_neuronx_cc_hook()
    partition_name = (
        nc.partition_id_tensor.name if nc.partition_id_tensor else None)
    in_names, out_names, out_avals, zero_outs = [], [], [], []
    for alloc in nc.m.functions[0].allocations:
        if not isinstance(alloc, mybir_.MemoryLocationSet):
            continue
        name = alloc.memorylocations[0].name
        if alloc.kind == "ExternalInput":
            if name != partition_name:
                in_names.append(name)
        elif alloc.kind == "ExternalOutput":
            shape = tuple(alloc.tensor_shape)
            dtype = mybir_.dt.np(alloc.dtype)
            out_names.append(name)
            out_avals.append(jax.core.ShapedArray(shape, dtype))
            zero_outs.append(np.zeros(shape, dtype))
    n_params = len(in_names)
    n_outs = len(out_avals)
    all_names = in_names + out_names
    if partition_name is not None:
        all_names = all_names + [partition_name]

    def _body(*args):
        operands = list(args)
        if partition_name is not None:
            operands.append(bass2jax.partition_id_tensor())
        outs = bass2jax._bass_exec_p.bind(
            *operands,
            out_avals=tuple(out_avals),
            in_names=tuple(all_names),
            out_names=tuple(out_names),
            lowering_input_output_aliases=(),
            sim_require_finite=True,
            sim_require_nnan=True,
            nc=nc,
        )
        return tuple(outs)

    devices = jax.devices()[:N_CORES]
    mesh = Mesh(np.asarray(devices), ("core",))
    donate = tuple(range(n_params, n_params + n_outs))
    sharded = jax.jit(
        shard_map(
            _body, mesh=mesh,
            in_specs=(PartitionSpec("core"),) * (n_params + n_outs),
            out_specs=(PartitionSpec("core"),) * n_outs,
            check_rep=False,
        ),
        donate_argnums=donate, keep_unused=True,
    )
    sharding = jax.sharding.NamedSharding(mesh, PartitionSpec("core"))
    concat_in = [
        jax.device_put(
            np.concatenate([np.asarray(in_maps[c][n]) for c in range(N_CORES)], 0),
            sharding)
        for n in in_names
    ]
    jax.block_until_ready(concat_in)

    def fresh_zeros():
        zs = [
            jax.device_put(
                np.zeros((N_CORES * z.shape[0], *z.shape[1:]), z.dtype), sharding)
            for z in zero_outs
        ]
        jax.block_until_ready(zs)
        return zs

    # warmup (compiles the jit)
    outs = sharded(*concat_in, *fresh_zeros())
    jax.block_until_ready(outs)

    times = []
    for _ in range(iters):
        zs = fresh_zeros()
        t0 = time.perf_counter()
        outs = sharded(*concat_in, *zs)
        jax.block_until_ready(outs)
        times.append(time.perf_counter() - t0)
    times = np.array(times)
    return {
        "min_s": float(times.min()),
        "median_s": float(np.median(times)),
        "mean_s": float(times.mean()),
        "all_s": times.tolist(),
    }


def kernel(x, cos, sin, Wq, Wk, Wv, Wo, _trace_flag=False):
    if "nc" not in _CACHE:
        _CACHE["nc"] = _build()
    nc = _CACHE["nc"]
    in_maps = _prep_inputs(
        np.asarray(x), np.asarray(cos), np.asarray(sin),
        np.asarray(Wq), np.asarray(Wk), np.asarray(Wv), np.asarray(Wo))
    res = run_bass_kernel_spmd(
        nc, in_maps, core_ids=list(range(N_CORES)), trace=_trace_flag)
    _CACHE["last_result"] = res
    out = np.empty((B, T, D), dtype=np.float32)
    for b in range(B):
        acc = res.results[4 * b]["out"].astype(np.float32).copy()
        for g in range(1, 4):
            acc += res.results[4 * b + g]["out"]
        out[b] = acc
    return out


# revision 11
# speedup vs baseline: 1.3306x; 1.3306x over previous
"""Causal self-attention (GQA, RoPE, QK-RMSNorm) Trainium2 Bass kernel.

Sharding: 8 cores = 2 batches x 4 KV groups. Core i handles batch i//4 and
KV group i%4 (4 query heads + 1 KV head). c_q/c_k/c_v column-sharded,
c_proj row-sharded; the partial output sums are reduced on the host.

Device-side layout trick: the host ships x^T (plus stacked cos/sin tables),
so every matmul contraction dim lands on SBUF partitions with zero on-device
transposes of x. Attention uses the "scores-transposed" formulation:
  s^T[tk, tq] = k^T.T @ q^T  (k block stationary)
  p^T = exp(s^T * scale)     (no max subtraction: QK-RMSNorm bounds |s*scale| <= sqrt(128))
  y[tq, d], r[tq] = p^T.T @ [v | 1]  (ones column gives the softmax denominator)
so softmax needs no partition-dim reductions and no probability transposes.
"""

import sys

sys.path.insert(0, "/opt/trn_rl_repo")

import numpy as np

import concourse.bacc as bacc
import concourse.tile as tile
import concourse.mybir as mybir
from concourse.bass_utils import run_bass_kernel_spmd

# Problem constants (hardcoded per contract)
B = 2
T = 2048
D = 2048
N_HEAD = 16
N_KV = 4
DH = 128
REP = N_HEAD // N_KV  # 4 query heads per KV head
HG = REP * DH  # 512 query columns per core
EPS = 1.1920928955078125e-07
SCALE = 1.0 / float(np.sqrt(DH))
MASK_VAL = -1e9

P = 128
TCH = 512  # T chunk (psum free dim)
NTCH = T // TCH  # 4
NDCH = D // P  # 16
NTB = T // P  # 16 Tk blocks

F32 = mybir.dt.float32
F32R = mybir.dt.float32r
BF16 = mybir.dt.bfloat16

N_CORES = 8

_CACHE = {}


def _build():
    nc = bacc.Bacc("TRN2", num_devices=N_CORES)
    xT = nc.dram_tensor("xT", [D, T], F32, kind="ExternalInput").ap()
    cos2 = nc.dram_tensor("cos2", [P, T], F32, kind="ExternalInput").ap()
    sin2n = nc.dram_tensor("sin2n", [P, T], F32, kind="ExternalInput").ap()
    wq = nc.dram_tensor("wq", [D, HG], F32, kind="ExternalInput").ap()
    wk = nc.dram_tensor("wk", [D, DH], F32, kind="ExternalInput").ap()
    wv = nc.dram_tensor("wv", [D, DH], F32, kind="ExternalInput").ap()
    wo = nc.dram_tensor("wo", [HG, D], F32, kind="ExternalInput").ap()
    out = nc.dram_tensor("out", [T, D], F32, kind="ExternalOutput").ap()

    with tile.TileContext(nc) as tc:
        _trace(tc, xT, cos2, sin2n, wq, wk, wv, wo, out)
    nc.compile()
    return nc


def _trace(tc, xT, cos2, sin2n, wq, wk, wv, wo, out):
    nc = tc.nc
    from contextlib import ExitStack

    ctx = ExitStack()
    with ctx:
        # ---------------- persistent pools ----------------
        const_pool = ctx.enter_context(tc.tile_pool(name="consts", bufs=1))
        attn_pool = ctx.enter_context(tc.tile_pool(name="attn", bufs=1))

        # constants
        ident_f = const_pool.tile([P, P], F32)
        nc.gpsimd.memset(ident_f, 0.0)
        nc.gpsimd.affine_select(
            out=ident_f, in_=ident_f, compare_op=mybir.AluOpType.not_equal,
            fill=1.0, base=0, pattern=[[-1, P]], channel_multiplier=1,
        )
        ident = const_pool.tile([P, P], F32R)
        nc.vector.tensor_copy(out=ident, in_=ident_f)

        onesm_f = const_pool.tile([P, P], F32)
        nc.vector.memset(onesm_f, 1.0)
        onesm = const_pool.tile([P, P], F32R)
        nc.vector.tensor_copy(out=onesm, in_=onesm_f)

        # additive causal mask for a diagonal 128x128 block: keep iff col >= row
        mask_sb = const_pool.tile([P, P], F32)
        nc.gpsimd.memset(mask_sb, 0.0)
        nc.gpsimd.affine_select(
            out=mask_sb, in_=mask_sb, compare_op=mybir.AluOpType.is_ge,
            fill=MASK_VAL, base=0, pattern=[[1, P]], channel_multiplier=-1,
        )

        eps_sb = const_pool.tile([P, 1], F32)
        nc.vector.memset(eps_sb, EPS)

        # final attention operands (fp32r / bf16)
        qrot = [attn_pool.tile([P, T], F32R, tag=f"qrot{h}", name=f"qrot{h}")
                for h in range(REP)]
        krot = attn_pool.tile([P, T], F32R, tag="krot", name="krot")
        vaug = [attn_pool.tile([P, DH + 1], BF16, tag=f"vaug{m}", name=f"vaug{m}")
                for m in range(NTB)]
        vsb = attn_pool.tile([P, T], F32R, tag="vsb", name="vsb")
        # ---------------- phases A-C: projections + RoPE/RMS + v prep ----------------
        # fused per T-chunk: evacuate proj psums into chunk tiles, do RMS stats
        # (all-ones matmul -> replicated column sums), RoPE via half-swap DMA,
        # and v transposes, all on [128, 512] slices.
        with tc.tile_pool(name="projw", bufs=1) as wpool, \
             tc.tile_pool(name="xts", bufs=4) as xpool, \
             tc.tile_pool(name="pre", bufs=2) as prepool, \
             tc.tile_pool(name="ps_proj", bufs=1, space="PSUM") as ps_proj, \
             tc.tile_pool(name="ps_rstd", bufs=2, space="PSUM") as ps_rstd:

            wq_sb = wpool.tile([P, NDCH, HG], F32R, name="wq_sb")
            wk_sb = wpool.tile([P, NDCH, DH], F32R, name="wk_sb")
            wv_sb = wpool.tile([P, NDCH, DH], F32R, name="wv_sb")
            wq_r = wq.rearrange("(n p) h -> p n h", p=P).bitcast(F32R)
            wk_r = wk.rearrange("(n p) h -> p n h", p=P).bitcast(F32R)
            wv_r = wv.rearrange("(n p) h -> p n h", p=P).bitcast(F32R)
            for s4 in range(0, NDCH, 4):
                nc.sync.dma_start(out=wq_sb[:, s4:s4 + 4, :], in_=wq_r[:, s4:s4 + 4, :])
                nc.sync.dma_start(out=wk_sb[:, s4:s4 + 4, :], in_=wk_r[:, s4:s4 + 4, :])
                nc.sync.dma_start(out=wv_sb[:, s4:s4 + 4, :], in_=wv_r[:, s4:s4 + 4, :])

            cos2_sb = wpool.tile([P, T], F32, name="cos2_sb")
            sin2n_sb = wpool.tile([P, T], F32, name="sin2n_sb")
            nc.sync.dma_start(out=cos2_sb, in_=cos2)
            nc.sync.dma_start(out=sin2n_sb, in_=sin2n)

            for j in range(NTCH):
                cs = slice(j * TCH, (j + 1) * TCH)
                psums = [ps_proj.tile([P, TCH], F32, tag=f"proj{i}", name=f"proj{i}_{j}")
                         for i in range(6)]
                for i in range(NDCH):
                    xt = xpool.tile([P, TCH], F32R, tag="xt", name=f"xt_{j}_{i}")
                    nc.sync.dma_start(
                        out=xt,
                        in_=xT[i * P:(i + 1) * P, j * TCH:(j + 1) * TCH].bitcast(F32R),
                    )
                    for h in range(REP):
                        nc.tensor.matmul(
                            psums[h], wq_sb[:, i, h * DH:(h + 1) * DH], xt,
                            start=(i == 0), stop=(i == NDCH - 1))
                    nc.tensor.matmul(psums[4], wk_sb[:, i, :], xt,
                                     start=(i == 0), stop=(i == NDCH - 1))
                    nc.tensor.matmul(psums[5], wv_sb[:, i, :], xt,
                                     start=(i == 0), stop=(i == NDCH - 1))

                # q/k heads: copy out, RMS stats, RoPE, write fp32r chunk
                for i in range(5):
                    dst = qrot[i] if i < REP else krot
                    pre = prepool.tile([P, TCH], F32, tag="pre", name=f"pre{i}_{j}")
                    nc.scalar.copy(out=pre, in_=psums[i])
                    sq = prepool.tile([P, TCH], F32R, tag="sq", name=f"sq{i}_{j}")
                    nc.vector.tensor_mul(sq, pre, pre)
                    rps = ps_rstd.tile([P, TCH], F32, tag="rstd", name=f"rstd{i}_{j}")
                    nc.tensor.matmul(rps, onesm, sq, start=True, stop=True)
                    srt = prepool.tile([P, TCH], F32, tag="srt", name=f"srt{i}_{j}")
                    nc.scalar.activation(
                        out=srt, in_=rps,
                        func=mybir.ActivationFunctionType.Sqrt,
                        scale=1.0 / DH, bias=eps_sb)
                    nc.vector.reciprocal(out=srt, in_=srt)
                    m1 = prepool.tile([P, TCH], F32, tag="m1", name=f"m1_{i}_{j}")
                    nc.vector.tensor_mul(m1, pre, cos2_sb[:, cs])
                    m2 = prepool.tile([P, TCH], F32, tag="m2", name=f"m2_{i}_{j}")
                    # rotate-half without a swap copy. Both DVE inputs must
                    # share a base partition (walrus samePartitionsAll); only
                    # the output base may differ. sin2n ships as [-sin; +sin].
                    nc.vector.tensor_mul(m2[0:64, :], pre[64:128, :],
                                         sin2n_sb[64:128, cs])
                    nc.vector.tensor_mul(m2[64:128, :], pre[0:64, :],
                                         sin2n_sb[0:64, cs])
                    nc.vector.tensor_add(m1, m1, m2)
                    nc.vector.tensor_mul(dst[:, cs], m1, srt)

                # v: evacuate chunk into the persistent v^T tile
                nc.scalar.copy(out=vsb[:, cs], in_=psums[5])

        # v transpose + ones column (own psum pool, released before phase D)
        with tc.tile_pool(name="ps_vtr", bufs=4, space="PSUM") as ps_vtr:
            for m in range(NTB):
                tr = ps_vtr.tile([P, P], F32R, tag="vtr", name=f"vtr{m}")
                nc.tensor.transpose(tr, vsb[:, m * P:(m + 1) * P], ident)
                nc.scalar.copy(out=vaug[m][:, 0:DH], in_=tr)
                nc.vector.memset(vaug[m][:, DH:DH + 1], 1.0)

        pt_pool = ctx.enter_context(tc.tile_pool(name="pt", bufs=2))
        yt_pool = ctx.enter_context(tc.tile_pool(name="yt", bufs=2))
        osb_pool = ctx.enter_context(tc.tile_pool(name="osb", bufs=3))
        ps_s = ctx.enter_context(tc.tile_pool(name="ps_s", bufs=3, space="PSUM"))
        ps_o = ctx.enter_context(tc.tile_pool(name="ps_o", bufs=2, space="PSUM"))
        ps_y = ctx.enter_context(tc.tile_pool(name="ps_y", bufs=2, space="PSUM"))
        ps_ytr = ctx.enter_context(tc.tile_pool(name="ps_ytr", bufs=1, space="PSUM"))

        # wo loads into space freed by the projection pools
        wo_pool = ctx.enter_context(tc.tile_pool(name="wop", bufs=1))
        wo_sb = wo_pool.tile([P, REP, D], F32R, name="wo_sb")
        nc.sync.dma_start(
            out=wo_sb,
            in_=wo.rearrange("(n p) d -> p n d", p=P).bitcast(F32R),
        )

        # ---------------- phase D: attention + output projection ----------------
        for j in range(NTCH):
            qs = slice(j * TCH, (j + 1) * TCH)
            yt = [yt_pool.tile([P, TCH], F32R, tag=f"yt{h}", name=f"yt{h}_{j}")
                  for h in range(REP)]
            for h in range(REP):
                pts = [pt_pool.tile([P, TCH], BF16, tag=f"pt{m}", name=f"pt{m}_{j}_{h}")
                       for m in range(4 * j + 4)]
                for m in range(4 * j + 4):
                    sps = ps_s.tile([P, TCH], F32, tag="s", name=f"s{j}_{h}_{m}")
                    nc.tensor.matmul(sps, krot[:, m * P:(m + 1) * P], qrot[h][:, qs],
                                     start=True, stop=True)
                    if m >= 4 * j:
                        dcol = P * (m - 4 * j)
                        ds_ = slice(dcol, dcol + P)
                        nc.vector.tensor_add(sps[:, ds_], sps[:, ds_], mask_sb)
                        # cols [0:dcol] are never read by any pv matmul
                        nc.scalar.activation(
                            out=pts[m][:, dcol:TCH], in_=sps[:, dcol:TCH],
                            func=mybir.ActivationFunctionType.Exp, scale=SCALE)
                    else:
                        nc.scalar.activation(
                            out=pts[m], in_=sps,
                            func=mybir.ActivationFunctionType.Exp, scale=SCALE)
                for n in range(4):
                    last = 4 * j + n
                    yps = ps_y.tile([P, DH + 1], F32, tag="y", name=f"y{j}_{h}_{n}")
                    for m in range(last + 1):
                        nc.tensor.matmul(yps, pts[m][:, n * P:(n + 1) * P], vaug[m],
                                         start=(m == 0), stop=(m == last))
                    rinv = osb_pool.tile([P, 1], F32, tag="rinv", name=f"rinv{j}{h}{n}")
                    nc.vector.reciprocal(out=rinv, in_=yps[:, DH:DH + 1])
                    ynorm = osb_pool.tile([P, P], F32R, tag="ynorm",
                                          name=f"ynorm{j}{h}{n}")
                    nc.vector.tensor_scalar_mul(ynorm, yps[:, 0:DH], rinv)
                    ytr = ps_ytr.tile([P, P], F32R, tag="ytr", name=f"ytr{j}{h}{n}")
                    nc.tensor.transpose(ytr, ynorm, ident)
                    nc.scalar.copy(out=yt[h][:, n * P:(n + 1) * P], in_=ytr)
            for n in range(4):
                osb = osb_pool.tile([P, D], F32, tag="osb", name=f"osb{j}{n}")
                for dc in range(NTCH):
                    ops = ps_o.tile([P, TCH], F32, tag="o", name=f"o{j}_{n}_{dc}")
                    for h in range(REP):
                        nc.tensor.matmul(
                            ops, yt[h][:, n * P:(n + 1) * P],
                            wo_sb[:, h, dc * TCH:(dc + 1) * TCH],
                            start=(h == 0), stop=(h == REP - 1))
                    nc.any.tensor_copy(
                        out=osb[:, dc * TCH:(dc + 1) * TCH], in_=ops)
                nc.sync.dma_start(
                    out=out[j * TCH + n * P: j * TCH + (n + 1) * P, :],
                    in_=osb)


def _prep_inputs(x, cos, sin, Wq, Wk, Wv, Wo):
    cosT = np.ascontiguousarray(cos[0, :, 0, :].T.astype(np.float32))  # [64, T]
    sinT = np.ascontiguousarray(sin[0, :, 0, :].T.astype(np.float32))
    cos2 = np.concatenate([cosT, cosT], axis=0)
    sin2n = np.concatenate([-sinT, sinT], axis=0)
    in_maps = []
    for i in range(N_CORES):
        b, g = i // 4, i % 4
        in_maps.append({
            "xT": np.ascontiguousarray(x[b].T.astype(np.float32)),
            "cos2": cos2,
            "sin2n": sin2n,
            "wq": np.ascontiguousarray(Wq[:, g * HG:(g + 1) * HG].astype(np.float32)),
            "wk": np.ascontiguousarray(Wk[:, g * DH:(g + 1) * DH].astype(np.float32)),
            "wv": np.ascontiguousarray(Wv[:, g * DH:(g + 1) * DH].astype(np.float32)),
            "wo": np.ascontiguousarray(Wo[g * HG:(g + 1) * HG, :].astype(np.float32)),
        })
    return in_maps


def bench(x, cos, sin, Wq, Wk, Wv, Wo, iters=20):
    """Device-resident timing of the compiled NEFF via the PJRT path.

    Stages all inputs (and fresh donated output buffers) on the devices
    before each timed call, so the measured wall time is dispatch + execute
    + sync only.
    """
    import time

    import jax
    from jax.sharding import Mesh, PartitionSpec
    from jax.experimental.shard_map import shard_map
    import concourse.bass2jax as bass2jax
    import concourse.mybir as mybir_

    if "nc" not in _CACHE:
        _CACHE["nc"] = _build()
    nc = _CACHE["nc"]
    in_maps = _prep_inputs(
        np.asarray(x), np.asarray(cos), np.asarray(sin),
        np.asarray(Wq), np.asarray(Wk), np.asarray(Wv), np.asarray(Wo))

    bass2jax.install_neuronx_cc_hook()
    partition_name = (
        nc.partition_id_tensor.name if nc.partition_id_tensor else None)
    in_names, out_names, out_avals, zero_outs = [], [], [], []
    for alloc in nc.m.functions[0].allocations:
        if not isinstance(alloc, mybir_.MemoryLocationSet):
            continue
        name = alloc.memorylocations[0].name
        if alloc.kind == "ExternalInput":
            if name != partition_name:
                in_names.append(name)
        elif alloc.kind == "ExternalOutput":
            shape = tuple(alloc.tensor_shape)
            dtype = mybir_.dt.np(alloc.dtype)
            out_names.append(name)
            out_avals.append(jax.core.ShapedArray(shape, dtype))
            zero_outs.append(np.zeros(shape, dtype))
    n_params = len(in_names)
    n_outs = len(out_avals)
    all_names = in_names + out_names
    if partition_name is not None:
        all_names = all_names + [partition_name]

    def _body(*args):
        operands = list(args)
        if partition_name is not None:
            operands.append(bass2jax.partition_id_tensor())
        outs = bass2jax._bass_exec_p.bind(
            *operands,
            out_avals=tuple(out_avals),
            in_names=tuple(all_names),
            out_names=tuple(out_names),
            lowering_input_output_aliases=(),
            sim_require_finite=True,
            sim_require_nnan=True,
            nc=nc,
        )
        return tuple(outs)

    devices = jax.devices()[:N_CORES]
    mesh = Mesh(np.asarray(devices), ("core",))
    donate = tuple(range(n_params, n_params + n_outs))
    sharded = jax.jit(
        shard_map(
            _body, mesh=mesh,
            in_specs=(PartitionSpec("core"),) * (n_params + n_outs),
            out_specs=(PartitionSpec("core"),) * n_outs,
            check_rep=False,
        ),
        donate_argnums=donate, keep_unused=True,
    )
    sharding = jax.sharding.NamedSharding(mesh, PartitionSpec("core"))
    concat_in = [
        jax.device_put(
            np.concatenate([np.asarray(in_maps[c][n]) for c in range(N_CORES)], 0),
            sharding)
        for n in in_names
    ]
    jax.block_until_ready(concat_in)

    def fresh_zeros():
        zs = [
            jax.device_put(
                np.zeros((N_CORES * z.shape[0], *z.shape[1:]), z.dtype), sharding)
            for z in zero_outs
        ]
        jax.block_until_ready(zs)
        return zs

    # warmup (compiles the jit)
    outs = sharded(*concat_in, *fresh_zeros())
    jax.block_until_ready(outs)

    times = []
    for _ in range(iters):
        zs = fresh_zeros()
        t0 = time.perf_counter()
        outs = sharded(*concat_in, *zs)
        jax.block_until_ready(outs)
        times.append(time.perf_counter() - t0)
    times = np.array(times)
    return {
        "min_s": float(times.min()),
        "median_s": float(np.median(times)),
        "mean_s": float(times.mean()),
        "all_s": times.tolist(),
    }


def kernel(x, cos, sin, Wq, Wk, Wv, Wo, _trace_flag=False):
    if "nc" not in _CACHE:
        _CACHE["nc"] = _build()
    nc = _CACHE["nc"]
    in_maps = _prep_inputs(
        np.asarray(x), np.asarray(cos), np.asarray(sin),
        np.asarray(Wq), np.asarray(Wk), np.asarray(Wv), np.asarray(Wo))
    res = run_bass_kernel_spmd(
        nc, in_maps, core_ids=list(range(N_CORES)), trace=_trace_flag)
    _CACHE["last_result"] = res
    out = np.empty((B, T, D), dtype=np.float32)
    for b in range(B):
        acc = res.results[4 * b]["out"].astype(np.float32).copy()
        for g in range(1, 4):
            acc += res.results[4 * b + g]["out"]
        out[b] = acc
    return out
